# revision 24
# baseline (speedup 1.0000x reference)
"""Trainium2 Bass kernel for the quantized ResNet bottleneck block.

Data-parallel over batch: 64 images -> 8 cores x 8 images.

v7: all matmuls single-pass bf16 (quantized activations are exact in bf16;
weights bf16-rounded, absorbed by the downstream bfp re-quantization).
Input/residual/output stream as bf16 (half the HBM traffic of fp32).
Weight-stationary matmul order: conv1 k-tiles / conv2 taps are loaded once
per 4-pixel-tile group and swept across 4 PSUM banks, so MMs stream at
~N cycles without per-MM LDWEIGHTS stalls.
  L1/L2: ACT bn+relu -> bf16 ygrp (2 images); DVE transpose-reduce 32-chan
         block max; 3-slice exponent math -> bf16 delta; 32x32 stream-
         transpose broadcast -> bf16 dcm; fused DVE round/clip quant.
  L3:    conv3 + identity-matmul residual (pixel-major bf16 xT) into 2-bank
         PSUM chunks; ACT relu-copy -> bf16 y3 frees PSUM; DVE block max +
         fused quant per 4-tile group; bf16 output DMA (exact).
bn3 trick: kernel consumes xb3 = x + bn3_beta; conv1 bias corrected by
-inv1 * (w1q_bf16 @ bn3_beta) on the host.  Host transposes/upcasts out.
"""
import numpy as np
import ml_dtypes
from contextlib import ExitStack

import concourse.bass as bass
import concourse.bacc as bacc
import concourse.tile as tile
from concourse import mybir
from concourse.bass_utils import run_bass_kernel_spmd

F32 = mybir.dt.float32
BF16 = mybir.dt.bfloat16
I32 = mybir.dt.int32
AL = mybir.AluOpType
AFT = mybir.ActivationFunctionType

# ---------------- custom DVE op: fused bfp round/clip/rescale ---------------
# out = min(max(in0 + in1*M, in1*M), in1*(M+127)) - in1*M
# with in1 = delta (power of two).  Adding M*delta rounds in0 to the delta
# grid (round-half-even); the clips implement relu and the 127 cap; the
# subtract is exact (Sterbenz).  M = 1.5 * 2^23.
import concourse.dve_ops as dve_ops
from concourse.dve_spec import Spec, Src0, Src1, C0, C1, minn, maxx

MAGIC = 12582912.0

def _bfp_ref(in0, in1, s0, s1, imm2):
    lo = in1 * s0
    return (np.minimum(np.maximum(in0 + lo, lo), in1 * s1) - lo).astype(np.float32)

BFP_QUANT_ANT = dve_ops.DveOp(
    "BFP_QUANT_ANT",
    Spec(
        body=minn(maxx(Src0 + Src1 * C0, Src1 * C0), Src1 * C1) - Src1 * C0,
        reference=_bfp_ref,
    ),
    subdim=False,
    uops_sha={"v3": "09229989be91bde3", "v4": "701a1ee7014b78c5"},
)

def _register_bfp_op():
    if "BFP_QUANT_ANT" not in dve_ops._SUB_OPCODE_FOR_NAME:
        dve_ops.OPS.append(BFP_QUANT_ANT)
        dve_ops.CUSTOM_DVE_SPECS["BFP_QUANT_ANT"] = BFP_QUANT_ANT.spec
        dve_ops._SUB_OPCODE_FOR_NAME["BFP_QUANT_ANT"] = (
            dve_ops._CUSTOM_DVE_ROW_BASE + len(dve_ops.OPS) - 1)

_register_bfp_op()

# ---------------- geometry (hardcoded for this problem) ---------------------
N_IMG = 8          # images per core
CIN = 512
WID = 128
H = W = 28
HW = H * W         # 784
PIX = N_IMG * HW   # 6272
PADH = PADW = 30
GRP = 1568         # quant group = 2 images
NT392 = 392        # conv N-tile (14 rows)


def _emit_delta_math(nc, rmax, out_bf):
    """rmax AP [128, nb] f32 -> delta = 2^(e-6), e=floor(log2(max(rmax,1e-24))),
    final mul retypes into a bf16 tile (delta is a power of two: exact)."""
    nc.vector.tensor_scalar_max(rmax, rmax, 1e-24)
    nc.vector.tensor_scalar(rmax.bitcast(I32), rmax.bitcast(I32),
                            0x7F800000, None, op0=AL.bitwise_and)
    nc.vector.tensor_scalar_mul(out_bf, rmax, 0.015625)


def build_nc():
    nc = bacc.Bacc()

    xb3 = nc.declare_dram_parameter("xb3", [N_IMG, CIN, HW], BF16, False)
    xTh = nc.declare_dram_parameter("xTh", [PIX, CIN], BF16, False)
    ident = nc.declare_dram_parameter("ident", [128, 128], BF16, False)
    w1T = nc.declare_dram_parameter("w1T", [CIN, WID], BF16, False)
    w2T = nc.declare_dram_parameter("w2T", [9, WID, WID], BF16, False)
    w3T = nc.declare_dram_parameter("w3T", [WID, CIN], BF16, False)
    inv1 = nc.declare_dram_parameter("inv1", [WID, 1], F32, False)
    bet1 = nc.declare_dram_parameter("bet1", [WID, 1], F32, False)
    inv2 = nc.declare_dram_parameter("inv2", [WID, 1], F32, False)
    bet2 = nc.declare_dram_parameter("bet2", [WID, 1], F32, False)
    outT = nc.declare_dram_parameter("outT", [PIX, CIN], BF16, True)

    with tile.TileContext(nc) as tc, ExitStack() as ctx:
        wp = ctx.enter_context(tc.tile_pool(name="wp", bufs=1))
        big = ctx.enter_context(tc.tile_pool(name="big", bufs=1))
        stage = ctx.enter_context(tc.tile_pool(name="stage", bufs=6))
        y3p = ctx.enter_context(tc.tile_pool(name="y3p", bufs=3))
        ygp = ctx.enter_context(tc.tile_pool(name="ygp", bufs=5))
        xs = ctx.enter_context(tc.tile_pool(name="xs", bufs=8))
        xt3 = ctx.enter_context(tc.tile_pool(name="xt3", bufs=4))
        dsm = ctx.enter_context(tc.tile_pool(name="dsm", bufs=6))
        trp = ctx.enter_context(tc.tile_pool(name="trp", bufs=2))
        pp = ctx.enter_context(tc.tile_pool(name="pp", bufs=4, space="PSUM"))
        p3p = ctx.enter_context(tc.tile_pool(name="p3p", bufs=2, space="PSUM"))

        # ---- params in ----
        w1sb = wp.tile([128, 4, WID], BF16)
        nc.sync.dma_start(w1sb[:], w1T[:].rearrange("(k c) o -> c k o", c=128))
        w2sb = wp.tile([128, 9, WID], BF16)
        nc.sync.dma_start(w2sb[:], w2T[:].rearrange("t c o -> c t o"))
        w3sb = wp.tile([128, CIN], BF16)
        nc.sync.dma_start(w3sb[:], w3T[:])
        identsb = wp.tile([128, 128], BF16)
        nc.sync.dma_start(identsb[:], ident[:])
        bn1s = wp.tile([128, 1], F32); nc.sync.dma_start(bn1s[:], inv1[:])
        bn1b = wp.tile([128, 1], F32); nc.sync.dma_start(bn1b[:], bet1[:])
        bn2s = wp.tile([128, 1], F32); nc.sync.dma_start(bn2s[:], inv2[:])
        bn2b = wp.tile([128, 1], F32); nc.sync.dma_start(bn2b[:], bet2[:])

        # ---- activations / residual state ----
        a1pad = big.tile([128, N_IMG, PADH, PADW], BF16)
        nc.gpsimd.memset(a1pad[:].rearrange("p n h w -> p (n h w)").bitcast(I32), 0)
        a2 = big.tile([128, PIX], BF16)

        # ================= emit functions =================
        taps = [(dy, dx) for dy in range(3) for dx in range(3)]

        def quant_grp(ygrp, emit_quants):
            rmax = dsm.tile([128, 49], F32, tag="rmax")
            nc.vector.tensor_reduce(rmax[:], ygrp[:].rearrange("p (b j) -> p b j", b=49, j=32),
                                    axis=mybir.AxisListType.X, op=AL.max,
                                    apply_transpose=True)
            rmaxb = dsm.tile([128, 49], BF16, tag="rmaxb")
            _emit_delta_math(nc, rmax[:], rmaxb[:])
            dcm = dsm.tile([128, GRP], BF16, tag="dcm")
            nc.vector.transpose(dcm[:], rmaxb[:].unsqueeze(2).broadcast_to([128, 49, 32]))
            emit_quants(ygrp, dcm)

        def emit_l1(g):
            ygrp = ygp.tile([128, GRP], BF16, tag="ygrp")
            xts = []
            for si in range(4):
                n = 2 * g + si // 2
                q0 = NT392 * (si % 2)
                xt = xs.tile([128, 4, NT392], BF16, tag="xk")
                nc.sync.dma_start(xt[:], xb3[n, :, q0:q0+NT392]
                                  .rearrange("(k c) q -> c k q", c=128))
                xts.append(xt)
            psts = [pp.tile([128, CIN], F32, tag="cp", name=f"ps1_{g}_{si}")
                    for si in range(4)]
            for k in range(4):                       # weight-stationary sweep
                for si in range(4):
                    nc.tensor.matmul(psts[si][:, :NT392], w1sb[:, k, :],
                                     xts[si][:, k, :],
                                     start=(k == 0), stop=(k == 3))
            for si in range(4):
                nc.scalar.activation(ygrp[:, si*NT392:(si+1)*NT392],
                                     psts[si][:, :NT392], AFT.Relu,
                                     bias=bn1b[:], scale=bn1s[:])

            def quants(ygrp, dcm):
                for im in range(2):
                    nc.vector._custom_dve(
                        BFP_QUANT_ANT,
                        out=a1pad[:, 2*g+im, 1:29, 1:29],
                        in0=ygrp[:, im*HW:(im+1)*HW],
                        in1=dcm[:, im*HW:(im+1)*HW],
                        s0=MAGIC, s1=MAGIC + 127.0,
                    )
            quant_grp(ygrp, quants)

        def emit_l2(g):
            ygrp = ygp.tile([128, GRP], BF16, tag="y2grp")
            psts = [pp.tile([128, CIN], F32, tag="cp", name=f"ps2_{g}_{si}")
                    for si in range(4)]
            for t, (dy, dx) in enumerate(taps):      # weight-stationary sweep
                for si in range(4):
                    n = 2 * g + si // 2
                    h0 = 14 * (si % 2)
                    rhs = a1pad[:, n, h0+dy:h0+dy+14, dx:dx+28]
                    nc.tensor.matmul(psts[si][:, :NT392], w2sb[:, t, :], rhs,
                                     start=(t == 0), stop=(t == 8))
            for si in range(4):
                nc.scalar.activation(ygrp[:, si*NT392:(si+1)*NT392],
                                     psts[si][:, :NT392], AFT.Relu,
                                     bias=bn2b[:], scale=bn2s[:])

            def quants(ygrp, dcm):
                nc.vector._custom_dve(
                    BFP_QUANT_ANT,
                    out=a2[:, 2*g*HW:(2*g+2)*HW],
                    in0=ygrp[:],
                    in1=dcm[:],
                    s0=MAGIC, s1=MAGIC + 127.0,
                )
            quant_grp(ygrp, quants)

        def emit_l3(t0, gn):
            """One L3 group: gn (<=4) 128-pixel tiles.  Loads and stores
            alternate between the two HWDGE rings (sync/scalar) so the
            store drain at the end of the kernel runs 2-wide."""
            ld = nc.sync if (t0 // 4) % 2 == 0 else nc.scalar
            st = nc.scalar if (t0 // 4) % 2 == 0 else nc.sync
            nf = gn * CIN
            xh = xt3.tile([128, 4 * CIN], BF16, tag="xh")
            ld.dma_start(xh[:, :nf].rearrange("p (j c) -> p j c", j=gn, c=CIN),
                              xTh[128*t0:128*t0 + 128*gn, :].rearrange("(j p) c -> p j c", p=128))
            y3 = y3p.tile([128, 4 * CIN], BF16, tag="y3")
            for c0 in range(0, gn, 2):
                cn = min(2, gn - c0)
                ps3 = p3p.tile([128, 2 * CIN], F32, tag="c3g")
                for j in range(c0, c0 + cn):
                    nc.tensor.matmul(ps3[:, (j-c0)*CIN:(j-c0+1)*CIN],
                                     a2[:, 128*(t0+j):128*(t0+j+1)], w3sb[:],
                                     start=True, stop=False)
                for j in range(c0, c0 + cn):
                    nc.tensor.matmul(ps3[:, (j-c0)*CIN:(j-c0+1)*CIN], identsb[:],
                                     xh[:, j*CIN:(j+1)*CIN],
                                     start=False, stop=True)
                nc.scalar.activation(y3[:, c0*CIN:(c0+cn)*CIN], ps3[:, :cn*CIN],
                                     AFT.Relu)
            nb = 16 * gn
            # block max via bf16 pairwise-max tree (2x DVE mode on packed bf16)
            yv = y3[:, :nf].rearrange("p (b k) -> p b k", k=32)
            tmp = trp.tile([128, 30 * 64], BF16, tag="trtmp")
            r16 = tmp[:, :nb*16].rearrange("p (b k) -> p b k", k=16)
            nc.vector.tensor_max(r16, yv[:, :, 0:16], yv[:, :, 16:32])
            r8 = tmp[:, 1024:1024+nb*8].rearrange("p (b k) -> p b k", k=8)
            nc.vector.tensor_max(r8, r16[:, :, 0:8], r16[:, :, 8:16])
            r4 = tmp[:, 1536:1536+nb*4].rearrange("p (b k) -> p b k", k=4)
            nc.vector.tensor_max(r4, r8[:, :, 0:4], r8[:, :, 4:8])
            r2 = tmp[:, 1792:1792+nb*2].rearrange("p (b k) -> p b k", k=2)
            nc.vector.tensor_max(r2, r4[:, :, 0:2], r4[:, :, 2:4])
            rm3 = dsm.tile([128, 64], F32, tag="rm3")
            nc.vector.tensor_max(rm3[:, :nb], r2[:, :, 0], r2[:, :, 1])
            rm3b = dsm.tile([128, 64], BF16, tag="rm3b")
            _emit_delta_math(nc, rm3[:, :nb], rm3b[:, :nb])
            o3 = stage.tile([128, 4 * CIN], BF16, tag="o3")
            nc.vector._custom_dve(
                BFP_QUANT_ANT,
                out=o3[:, :nf].rearrange("p (b k) -> p b k", k=32),
                in0=y3[:, :nf].rearrange("p (b k) -> p b k", k=32),
                in1=rm3b[:, :nb].unsqueeze(2).broadcast_to([128, nb, 32]),
                s0=MAGIC, s1=MAGIC + 127.0,
            )
            # split the store across both HWDGE rings: halves drain in parallel
            g0 = (gn + 1) // 2
            st.dma_start(outT[128*t0:128*t0 + 128*g0, :].rearrange("(j p) c -> p j c", p=128),
                         o3[:, :g0*CIN].rearrange("p (j c) -> p j c", j=g0, c=CIN))
            if gn > g0:
                ld.dma_start(outT[128*(t0+g0):128*t0 + 128*gn, :].rearrange("(j p) c -> p j c", p=128),
                             o3[:, g0*CIN:nf].rearrange("p (j c) -> p j c", j=gn-g0, c=CIN))

        # ================= interleaved schedule =================
        l3g = [(4*i, min(4, 49 - 4*i)) for i in range((49 + 3) // 4)]  # 13 groups
        emit_l1(0)
        emit_l1(1)
        emit_l1(2)
        emit_l2(0)
        emit_l1(3)
        for t0, gn in l3g[:3]:      # tiles 0-11, needs quant2(0) only
            emit_l3(t0, gn)
        emit_l2(1)
        for t0, gn in l3g[3:6]:     # tiles 12-23, needs quant2(1)
            emit_l3(t0, gn)
        emit_l2(2)
        for t0, gn in l3g[6:9]:     # tiles 24-35, needs quant2(2)
            emit_l3(t0, gn)
        emit_l2(3)
        for t0, gn in l3g[9:]:      # tiles 36-48
            emit_l3(t0, gn)

    nc.finalize()
    return nc


# ---------------- host-side parameter prep ---------------------------------
def _w_quant_np(w, blk=32):
    O, I, kh, kw = w.shape
    wb = w.reshape(O, I // blk, blk, kh, kw)
    alpha = np.maximum(np.abs(wb).max(axis=2, keepdims=True) / np.float32(127.0),
                       np.float32(1e-24)).astype(np.float32)
    q = (np.round(wb / alpha) * alpha).astype(np.float32)
    return q.reshape(O, I, kh, kw)


def _bn_fold(g, b, m, v):
    inv = (g / np.sqrt(v + np.float32(1e-5))).astype(np.float32)
    beta = (b - m * inv).astype(np.float32)
    return inv, beta


_NC_CACHE = {}

def kernel(x, w1, w2, w3,
           bn1_g, bn1_b, bn1_m, bn1_v,
           bn2_g, bn2_b, bn2_m, bn2_v,
           bn3_g, bn3_b, bn3_m, bn3_v,
           _want_trace=False):
    x = np.asarray(x, np.float32)
    w1q = _w_quant_np(np.asarray(w1, np.float32))
    w2q = _w_quant_np(np.asarray(w2, np.float32))
    w3q = _w_quant_np(np.asarray(w3, np.float32))
    inv1, bet1 = _bn_fold(*[np.asarray(a, np.float32) for a in (bn1_g, bn1_b, bn1_m, bn1_v)])
    inv2, bet2 = _bn_fold(*[np.asarray(a, np.float32) for a in (bn2_g, bn2_b, bn2_m, bn2_v)])
    inv3, bet3 = _bn_fold(*[np.asarray(a, np.float32) for a in (bn3_g, bn3_b, bn3_m, bn3_v)])

    # bn3 beta folded into the residual input; conv1 bias corrected for it
    xb3 = (x + bet3[None, :, None, None]).astype(np.float32)

    w1b = w1q[:, :, 0, 0].astype(ml_dtypes.bfloat16)            # [128, 512]
    K = (w1b.astype(np.float64) @ bet3.astype(np.float64))
    bet1c = (bet1.astype(np.float64) - inv1.astype(np.float64) * K).astype(np.float32)

    w1T = np.ascontiguousarray(w1b.T)                                      # [512, 128] bf16
    w2T = np.ascontiguousarray(
        w2q.transpose(2, 3, 1, 0).reshape(9, WID, WID)).astype(ml_dtypes.bfloat16)
    w3f = (w3q[:, :, 0, 0] * inv3[:, None]).astype(np.float32)
    w3T = np.ascontiguousarray(w3f.T).astype(ml_dtypes.bfloat16)           # [128, 512] bf16

    xb3v = xb3.reshape(64, CIN, HW).astype(ml_dtypes.bfloat16)
    xTv = np.ascontiguousarray(
        xb3.reshape(64, CIN, HW).transpose(0, 2, 1)).astype(ml_dtypes.bfloat16)

    if "nc" not in _NC_CACHE:
        _NC_CACHE["nc"] = build_nc()
    nc = _NC_CACHE["nc"]

    shared = dict(
        w1T=w1T, w2T=w2T, w3T=w3T,
        ident=np.eye(128, dtype=ml_dtypes.bfloat16),
        inv1=inv1.reshape(WID, 1), bet1=bet1c.reshape(WID, 1),
        inv2=inv2.reshape(WID, 1), bet2=bet2.reshape(WID, 1),
    )
    in_maps = []
    for c in range(8):
        m = dict(shared)
        m["xb3"] = np.ascontiguousarray(xb3v[8*c:8*(c+1)])
        m["xTh"] = np.ascontiguousarray(xTv[8*c:8*(c+1)].reshape(PIX, CIN))
        in_maps.append(m)

    res = run_bass_kernel_spmd(nc, in_maps, list(range(8)), trace=_want_trace)
    out = np.empty((64, CIN, H, W), np.float32)
    for c in range(8):
        oT = res.results[c]["outT"].astype(np.float32).reshape(N_IMG, HW, CIN)
        out[8*c:8*(c+1)] = oT.transpose(0, 2, 1).reshape(N_IMG, CIN, H, W)
    if _want_trace:
        return out, res
    return out


# revision 31
# speedup vs baseline: 1.0300x; 1.0300x over previous
"""Trainium2 Bass kernel for the quantized ResNet bottleneck block.

Data-parallel over batch: 64 images -> 8 cores x 8 images.

v7: all matmuls single-pass bf16 (quantized activations are exact in bf16;
weights bf16-rounded, absorbed by the downstream bfp re-quantization).
Input/residual/output stream as bf16 (half the HBM traffic of fp32).
Weight-stationary matmul order: conv1 k-tiles / conv2 taps are loaded once
per 4-pixel-tile group and swept across 4 PSUM banks, so MMs stream at
~N cycles without per-MM LDWEIGHTS stalls.
  L1/L2: ACT bn+relu -> bf16 ygrp (2 images); DVE transpose-reduce 32-chan
         block max; 3-slice exponent math -> bf16 delta; 32x32 stream-
         transpose broadcast -> bf16 dcm; fused DVE round/clip quant.
  L3:    conv3 + identity-matmul residual (pixel-major bf16 xT) into 2-bank
         PSUM chunks; ACT relu-copy -> bf16 y3 frees PSUM; DVE block max +
         fused quant per 4-tile group; bf16 output DMA (exact).
bn3 trick: kernel consumes xb3 = x + bn3_beta; conv1 bias corrected by
-inv1 * (w1q_bf16 @ bn3_beta) on the host.  Host transposes/upcasts out.
"""
import numpy as np
import ml_dtypes
from contextlib import ExitStack

import concourse.bass as bass
import concourse.bacc as bacc
import concourse.tile as tile
from concourse import mybir
from concourse.bass_utils import run_bass_kernel_spmd

F32 = mybir.dt.float32
BF16 = mybir.dt.bfloat16
I32 = mybir.dt.int32
AL = mybir.AluOpType
AFT = mybir.ActivationFunctionType

# ---------------- custom DVE op: fused bfp round/clip/rescale ---------------
# out = min(max(in0 + in1*M, in1*M), in1*(M+127)) - in1*M
# with in1 = delta (power of two).  Adding M*delta rounds in0 to the delta
# grid (round-half-even); the clips implement relu and the 127 cap; the
# subtract is exact (Sterbenz).  M = 1.5 * 2^23.
import concourse.dve_ops as dve_ops
from concourse.dve_spec import Spec, Src0, Src1, C0, C1, minn, maxx

MAGIC = 12582912.0

def _bfp_ref(in0, in1, s0, s1, imm2):
    lo = in1 * s0
    return (np.minimum(np.maximum(in0 + lo, lo), in1 * s1) - lo).astype(np.float32)

BFP_QUANT_ANT = dve_ops.DveOp(
    "BFP_QUANT_ANT",
    Spec(
        body=minn(maxx(Src0 + Src1 * C0, Src1 * C0), Src1 * C1) - Src1 * C0,
        reference=_bfp_ref,
    ),
    subdim=False,
    uops_sha={"v3": "09229989be91bde3", "v4": "701a1ee7014b78c5"},
)

def _register_bfp_op():
    if "BFP_QUANT_ANT" not in dve_ops._SUB_OPCODE_FOR_NAME:
        dve_ops.OPS.append(BFP_QUANT_ANT)
        dve_ops.CUSTOM_DVE_SPECS["BFP_QUANT_ANT"] = BFP_QUANT_ANT.spec
        dve_ops._SUB_OPCODE_FOR_NAME["BFP_QUANT_ANT"] = (
            dve_ops._CUSTOM_DVE_ROW_BASE + len(dve_ops.OPS) - 1)

_register_bfp_op()

# ---------------- geometry (hardcoded for this problem) ---------------------
N_IMG = 8          # images per core
CIN = 512
WID = 128
H = W = 28
HW = H * W         # 784
PIX = N_IMG * HW   # 6272
PADH = PADW = 30
GRP = 1568         # quant group = 2 images
NT392 = 392        # conv N-tile (14 rows)


def _emit_delta_math(nc, rmax, out_bf):
    """rmax AP [128, nb] f32 -> delta = 2^(e-6), e=floor(log2(max(rmax,1e-24))),
    final mul retypes into a bf16 tile (delta is a power of two: exact)."""
    nc.vector.tensor_scalar_max(rmax, rmax, 1e-24)
    nc.vector.tensor_scalar(rmax.bitcast(I32), rmax.bitcast(I32),
                            0x7F800000, None, op0=AL.bitwise_and)
    nc.vector.tensor_scalar_mul(out_bf, rmax, 0.015625)


def build_nc():
    nc = bacc.Bacc()

    xb3 = nc.declare_dram_parameter("xb3", [N_IMG, CIN, HW], BF16, False)
    xTh = nc.declare_dram_parameter("xTh", [PIX, CIN], BF16, False)
    pkb = nc.declare_dram_parameter("pkb", [128, 1152], BF16, False)
    w2T = nc.declare_dram_parameter("w2T", [9, WID, WID], BF16, False)
    pkf = nc.declare_dram_parameter("pkf", [128, 4], F32, False)
    outT = nc.declare_dram_parameter("outT", [PIX, CIN], BF16, True)

    with tile.TileContext(nc) as tc, ExitStack() as ctx:
        wp = ctx.enter_context(tc.tile_pool(name="wp", bufs=1))
        big = ctx.enter_context(tc.tile_pool(name="big", bufs=1))
        stage = ctx.enter_context(tc.tile_pool(name="stage", bufs=6))
        y3p = ctx.enter_context(tc.tile_pool(name="y3p", bufs=3))
        ygp = ctx.enter_context(tc.tile_pool(name="ygp", bufs=5))
        xs = ctx.enter_context(tc.tile_pool(name="xs", bufs=8))
        xt3 = ctx.enter_context(tc.tile_pool(name="xt3", bufs=4))
        dsm = ctx.enter_context(tc.tile_pool(name="dsm", bufs=6))
        trp = ctx.enter_context(tc.tile_pool(name="trp", bufs=2))
        pp = ctx.enter_context(tc.tile_pool(name="pp", bufs=4, space="PSUM"))
        p3p = ctx.enter_context(tc.tile_pool(name="p3p", bufs=2, space="PSUM"))

        # ---- params in (packed: 3 DMAs total) ----
        pkbs = wp.tile([128, 1152], BF16)
        nc.sync.dma_start(pkbs[:], pkb[:])
        w2sb = wp.tile([128, 9, WID], BF16)
        nc.scalar.dma_start(w2sb[:], w2T[:].rearrange("t c o -> c t o"))
        pkfs = wp.tile([128, 4], F32)
        nc.sync.dma_start(pkfs[:], pkf[:])
        w1sb = pkbs[:, 0:512].rearrange("p (k o) -> p k o", k=4)   # [128, 4, 128] view
        w3sb = pkbs[:, 512:1024]
        identsb = pkbs[:, 1024:1152]
        bn1s = pkfs[:, 0:1]; bn1b = pkfs[:, 1:2]
        bn2s = pkfs[:, 2:3]; bn2b = pkfs[:, 3:4]

        # ---- activations / residual state ----
        a1pad = big.tile([128, N_IMG, PADH, PADW], BF16)
        nc.gpsimd.memset(a1pad[:].rearrange("p n h w -> p (n h w)").bitcast(I32), 0)
        a2 = big.tile([128, PIX], BF16)

        # ================= emit functions =================
        taps = [(dy, dx) for dy in range(3) for dx in range(3)]

        def quant_grp(ygrp, emit_quants):
            rmax = dsm.tile([128, 49], F32, tag="rmax")
            nc.vector.tensor_reduce(rmax[:], ygrp[:].rearrange("p (b j) -> p b j", b=49, j=32),
                                    axis=mybir.AxisListType.X, op=AL.max,
                                    apply_transpose=True)
            rmaxb = dsm.tile([128, 49], BF16, tag="rmaxb")
            _emit_delta_math(nc, rmax[:], rmaxb[:])
            dcm = dsm.tile([128, GRP], BF16, tag="dcm")
            nc.vector.transpose(dcm[:], rmaxb[:].unsqueeze(2).broadcast_to([128, 49, 32]))
            emit_quants(ygrp, dcm)

        def emit_l1(g):
            ygrp = ygp.tile([128, GRP], BF16, tag="ygrp")
            xts = []
            for si in range(4):
                n = 2 * g + si // 2
                q0 = NT392 * (si % 2)
                xt = xs.tile([128, 4, NT392], BF16, tag="xk")
                nc.sync.dma_start(xt[:], xb3[n, :, q0:q0+NT392]
                                  .rearrange("(k c) q -> c k q", c=128))
                xts.append(xt)
            psts = [pp.tile([128, CIN], F32, tag="cp", name=f"ps1_{g}_{si}")
                    for si in range(4)]
            for k in range(4):                       # weight-stationary sweep
                for si in range(4):
                    nc.tensor.matmul(psts[si][:, :NT392], w1sb[:, k, :],
                                     xts[si][:, k, :],
                                     start=(k == 0), stop=(k == 3))
            for si in range(4):
                nc.scalar.activation(ygrp[:, si*NT392:(si+1)*NT392],
                                     psts[si][:, :NT392], AFT.Relu,
                                     bias=bn1b, scale=bn1s)

            def quants(ygrp, dcm):
                for im in range(2):
                    nc.vector._custom_dve(
                        BFP_QUANT_ANT,
                        out=a1pad[:, 2*g+im, 1:29, 1:29],
                        in0=ygrp[:, im*HW:(im+1)*HW],
                        in1=dcm[:, im*HW:(im+1)*HW],
                        s0=MAGIC, s1=MAGIC + 127.0,
                    )
            quant_grp(ygrp, quants)

        def emit_l2(g):
            ygrp = ygp.tile([128, GRP], BF16, tag="y2grp")
            psts = [pp.tile([128, CIN], F32, tag="cp", name=f"ps2_{g}_{si}")
                    for si in range(4)]
            for t, (dy, dx) in enumerate(taps):      # weight-stationary sweep
                for si in range(4):
                    n = 2 * g + si // 2
                    h0 = 14 * (si % 2)
                    rhs = a1pad[:, n, h0+dy:h0+dy+14, dx:dx+28]
                    nc.tensor.matmul(psts[si][:, :NT392], w2sb[:, t, :], rhs,
                                     start=(t == 0), stop=(t == 8))
            for si in range(4):
                nc.scalar.activation(ygrp[:, si*NT392:(si+1)*NT392],
                                     psts[si][:, :NT392], AFT.Relu,
                                     bias=bn2b, scale=bn2s)

            def quants(ygrp, dcm):
                nc.vector._custom_dve(
                    BFP_QUANT_ANT,
                    out=a2[:, 2*g*HW:(2*g+2)*HW],
                    in0=ygrp[:],
                    in1=dcm[:],
                    s0=MAGIC, s1=MAGIC + 127.0,
                )
            quant_grp(ygrp, quants)

        def emit_l3(t0, gn):
            """One L3 group: gn (<=4) 128-pixel tiles.  Loads and stores
            alternate between the two HWDGE rings (sync/scalar) so the
            store drain at the end of the kernel runs 2-wide."""
            ld = nc.sync if (t0 // 4) % 2 == 0 else nc.scalar
            st = nc.scalar if (t0 // 4) % 2 == 0 else nc.sync
            nf = gn * CIN
            xh = xt3.tile([128, 4 * CIN], BF16, tag="xh")
            ld.dma_start(xh[:, :nf].rearrange("p (j c) -> p j c", j=gn, c=CIN),
                              xTh[128*t0:128*t0 + 128*gn, :].rearrange("(j p) c -> p j c", p=128))
            y3 = y3p.tile([128, 4 * CIN], BF16, tag="y3")
            for c0 in range(0, gn, 2):
                cn = min(2, gn - c0)
                ps3 = p3p.tile([128, 2 * CIN], F32, tag="c3g")
                for j in range(c0, c0 + cn):
                    nc.tensor.matmul(ps3[:, (j-c0)*CIN:(j-c0+1)*CIN],
                                     a2[:, 128*(t0+j):128*(t0+j+1)], w3sb,
                                     start=True, stop=False)
                for j in range(c0, c0 + cn):
                    nc.tensor.matmul(ps3[:, (j-c0)*CIN:(j-c0+1)*CIN], identsb,
                                     xh[:, j*CIN:(j+1)*CIN],
                                     start=False, stop=True)
                nc.scalar.activation(y3[:, c0*CIN:(c0+cn)*CIN], ps3[:, :cn*CIN],
                                     AFT.Relu)
            nb = 16 * gn
            # block max via bf16 pairwise-max tree (2x DVE mode on packed bf16)
            yv = y3[:, :nf].rearrange("p (b k) -> p b k", k=32)
            tmp = trp.tile([128, 30 * 64], BF16, tag="trtmp")
            r16 = tmp[:, :nb*16].rearrange("p (b k) -> p b k", k=16)
            nc.vector.tensor_max(r16, yv[:, :, 0:16], yv[:, :, 16:32])
            r8 = tmp[:, 1024:1024+nb*8].rearrange("p (b k) -> p b k", k=8)
            nc.vector.tensor_max(r8, r16[:, :, 0:8], r16[:, :, 8:16])
            r4 = tmp[:, 1536:1536+nb*4].rearrange("p (b k) -> p b k", k=4)
            nc.vector.tensor_max(r4, r8[:, :, 0:4], r8[:, :, 4:8])
            r2 = tmp[:, 1792:1792+nb*2].rearrange("p (b k) -> p b k", k=2)
            nc.vector.tensor_max(r2, r4[:, :, 0:2], r4[:, :, 2:4])
            rm3 = dsm.tile([128, 64], F32, tag="rm3")
            nc.vector.tensor_max(rm3[:, :nb], r2[:, :, 0], r2[:, :, 1])
            rm3b = dsm.tile([128, 64], BF16, tag="rm3b")
            _emit_delta_math(nc, rm3[:, :nb], rm3b[:, :nb])
            o3 = stage.tile([128, 4 * CIN], BF16, tag="o3")
            nc.vector._custom_dve(
                BFP_QUANT_ANT,
                out=o3[:, :nf].rearrange("p (b k) -> p b k", k=32),
                in0=y3[:, :nf].rearrange("p (b k) -> p b k", k=32),
                in1=rm3b[:, :nb].unsqueeze(2).broadcast_to([128, nb, 32]),
                s0=MAGIC, s1=MAGIC + 127.0,
            )
            st.dma_start(outT[128*t0:128*t0 + 128*gn, :].rearrange("(j p) c -> p j c", p=128),
                         o3[:, :nf].rearrange("p (j c) -> p j c", j=gn, c=CIN))

        # ================= interleaved schedule =================
        l3g = [(4*i, min(4, 49 - 4*i)) for i in range((49 + 3) // 4)]  # 13 groups
        emit_l1(0)
        emit_l1(1)
        emit_l1(2)
        emit_l2(0)
        emit_l1(3)
        for t0, gn in l3g[:3]:      # tiles 0-11, needs quant2(0) only
            emit_l3(t0, gn)
        emit_l2(1)
        for t0, gn in l3g[3:6]:     # tiles 12-23, needs quant2(1)
            emit_l3(t0, gn)
        emit_l2(2)
        for t0, gn in l3g[6:9]:     # tiles 24-35, needs quant2(2)
            emit_l3(t0, gn)
        emit_l2(3)
        for t0, gn in l3g[9:]:      # tiles 36-48
            emit_l3(t0, gn)

    nc.finalize()
    return nc


# ---------------- host-side parameter prep ---------------------------------
def _w_quant_np(w, blk=32):
    O, I, kh, kw = w.shape
    wb = w.reshape(O, I // blk, blk, kh, kw)
    alpha = np.maximum(np.abs(wb).max(axis=2, keepdims=True) / np.float32(127.0),
                       np.float32(1e-24)).astype(np.float32)
    q = (np.round(wb / alpha) * alpha).astype(np.float32)
    return q.reshape(O, I, kh, kw)


def _bn_fold(g, b, m, v):
    inv = (g / np.sqrt(v + np.float32(1e-5))).astype(np.float32)
    beta = (b - m * inv).astype(np.float32)
    return inv, beta


_NC_CACHE = {}

def kernel(x, w1, w2, w3,
           bn1_g, bn1_b, bn1_m, bn1_v,
           bn2_g, bn2_b, bn2_m, bn2_v,
           bn3_g, bn3_b, bn3_m, bn3_v,
           _want_trace=False):
    x = np.asarray(x, np.float32)
    w1q = _w_quant_np(np.asarray(w1, np.float32))
    w2q = _w_quant_np(np.asarray(w2, np.float32))
    w3q = _w_quant_np(np.asarray(w3, np.float32))
    inv1, bet1 = _bn_fold(*[np.asarray(a, np.float32) for a in (bn1_g, bn1_b, bn1_m, bn1_v)])
    inv2, bet2 = _bn_fold(*[np.asarray(a, np.float32) for a in (bn2_g, bn2_b, bn2_m, bn2_v)])
    inv3, bet3 = _bn_fold(*[np.asarray(a, np.float32) for a in (bn3_g, bn3_b, bn3_m, bn3_v)])

    # bn3 beta folded into the residual input; conv1 bias corrected for it
    xb3 = (x + bet3[None, :, None, None]).astype(np.float32)

    w1b = w1q[:, :, 0, 0].astype(ml_dtypes.bfloat16)            # [128, 512]
    K = (w1b.astype(np.float64) @ bet3.astype(np.float64))
    bet1c = (bet1.astype(np.float64) - inv1.astype(np.float64) * K).astype(np.float32)

    w2T = np.ascontiguousarray(
        w2q.transpose(2, 3, 1, 0).reshape(9, WID, WID)).astype(ml_dtypes.bfloat16)
    w3f = (w3q[:, :, 0, 0] * inv3[:, None]).astype(np.float32)
    w3T = np.ascontiguousarray(w3f.T).astype(ml_dtypes.bfloat16)           # [128, 512] bf16

    # packed bf16 params: [w1 (c,k,o) | w3T | ident]  -> [128, 1152]
    pkb = np.empty((128, 1152), ml_dtypes.bfloat16)
    pkb[:, 0:512] = w1b.reshape(WID, 4, 128).transpose(2, 1, 0).reshape(128, 512)
    pkb[:, 512:1024] = w3T
    pkb[:, 1024:1152] = np.eye(128, dtype=ml_dtypes.bfloat16)
    pkf = np.stack([inv1, bet1c, inv2, bet2], axis=1).astype(np.float32)   # [128, 4]

    xb3v = xb3.reshape(64, CIN, HW).astype(ml_dtypes.bfloat16)
    xTv = np.ascontiguousarray(
        xb3.reshape(64, CIN, HW).transpose(0, 2, 1)).astype(ml_dtypes.bfloat16)

    if "nc" not in _NC_CACHE:
        _NC_CACHE["nc"] = build_nc()
    nc = _NC_CACHE["nc"]

    shared = dict(pkb=pkb, w2T=w2T, pkf=pkf)
    in_maps = []
    for c in range(8):
        m = dict(shared)
        m["xb3"] = np.ascontiguousarray(xb3v[8*c:8*(c+1)])
        m["xTh"] = np.ascontiguousarray(xTv[8*c:8*(c+1)].reshape(PIX, CIN))
        in_maps.append(m)

    res = run_bass_kernel_spmd(nc, in_maps, list(range(8)), trace=_want_trace)
    out = np.empty((64, CIN, H, W), np.float32)
    for c in range(8):
        oT = res.results[c]["outT"].astype(np.float32).reshape(N_IMG, HW, CIN)
        out[8*c:8*(c+1)] = oT.transpose(0, 2, 1).reshape(N_IMG, CIN, H, W)
    if _want_trace:
        return out, res
    return out
